# revision 15
# baseline (speedup 1.0000x reference)
"""Trainium2 Bass kernel for nn_DeepSet (segment_reduce DeepSet over 3 node types).

Strategy (8 NeuronCores, SPMD):
  - Host: only nodes whose segment id is in [0, B) contribute to segment ops
    (jax drops out-of-range scatter ids); a graph's output row is finite only
    if all three node types have >=1 valid node there.  Active graphs are
    round-robin assigned to the 8 cores.  Per core and node type, graphs are
    padded to a fixed length L; pad slots replicate the graph's FIRST node so
    segment max stays exact and segment sum is fixed with a rank-1 correction.
  - Device layout: 8 graph-chunks x 16 features packed on 128 partitions
    (feature-major, nodes along the free dim).  All MLP layers are single
    matmuls with block-diagonal weights; segment stats are strided
    tensor_reduce ops; per-graph stat projections are broadcast back with
    stride-0 access patterns.  Inactive rows of the output are NaN (matching
    the reference's 0/0 means).
"""
import sys

sys.path.insert(0, "/opt/trn_rl_repo")

import numpy as np

import concourse.bass as bass
import concourse.tile as tile
from concourse import mybir
from concourse.bass_utils import run_bass_kernel_spmd

N_CORES = 8
HS = 16
NMOD = 3
F32 = mybir.dt.float32
PSUM_COLS = 512

_PROGRAM_CACHE = {}
LAST_EXEC_NS = None


# ----------------------------------------------------------------------------
# host-side prep
# ----------------------------------------------------------------------------

def _blkdiag(w):
    """[16,16] -> [128,128] block diagonal (chunk-major packing)."""
    return np.kron(np.eye(8, dtype=np.float32), np.asarray(w, np.float32))


def _inf_blk(w, f):
    """[F,16] -> [8F,128]: row 8*fi+c, cols 16c:16c+16 = w[fi,:] (plane-major raw)."""
    out = np.zeros((8 * f, 128), np.float32)
    w = np.asarray(w, np.float32)
    for c in range(8):
        for fi in range(f):
            out[8 * fi + c, 16 * c:16 * c + 16] = w[fi]
    return out


def _rep16(v):
    """[G] -> [128, G] replicated across the 16 feature rows of every chunk."""
    return np.tile(np.asarray(v, np.float32)[None, :], (128, 1))


def _b_rep(b):
    """[16] bias -> [128,1] chunk-replicated."""
    return np.tile(np.asarray(b, np.float32), 8)[:, None]


class TypePrep:
    def __init__(self, name, kind, feats, seg, B, gids_per_core):
        self.name, self.kind, self.F = name, kind, len(feats)
        seg = np.asarray(seg).astype(np.int64)
        valid = (seg >= 0) & (seg < B)
        vidx = np.nonzero(valid)[0]
        order = vidx[np.argsort(seg[vidx], kind="stable")]
        cnt_all = np.bincount(seg[vidx], minlength=B)
        starts_all = np.concatenate([[0], np.cumsum(cnt_all)]).astype(np.int64)
        self.L = max(8, int(max((cnt_all[g].max() if len(g) else 0)
                                for g in gids_per_core)))
        self.feats = [np.asarray(f, np.float32) for f in feats]
        self.order, self.cnt_all, self.starts_all = order, cnt_all, starts_all

    def core_tables(self, gids, G):
        L = self.L
        idx = np.zeros((G, L), np.int64)
        cnt = np.zeros(G, np.int64)
        fallback = self.order[0] if len(self.order) else 0
        idx[:, :] = fallback
        for k, g in enumerate(gids):
            s, n = self.starts_all[g], self.cnt_all[g]
            nodes = self.order[s:s + n]
            idx[k, :n] = nodes
            idx[k, n:] = nodes[0]
            cnt[k] = n
        return idx, cnt

    def raw_packed(self, idx, G):
        """[8F, C] plane-major packed raw features. chunk c = graphs [c*Gc,(c+1)*Gc)."""
        L, F = self.L, self.F
        Gc = G // 8
        C = Gc * L
        out = np.empty((8 * F, C), np.float32)
        for fi, f in enumerate(self.feats):
            v = f[idx]                       # [G, L]
            v = v.reshape(8, Gc * L)         # chunk-major
            out[8 * fi:8 * fi + 8, :] = v
        return out


def _host_prep(inputs):
    B = int(np.asarray(inputs["glob_E"]).shape[0])
    cc = np.asarray(inputs["cells_center"], np.float32)
    tcn = np.asarray(inputs["tracks_center"], np.float32)
    sc = np.asarray(inputs["subjets_center"], np.float32)

    segs = {t: np.asarray(inputs[t + "_seg"]).astype(np.int64)
            for t in ("cells", "tracks", "subjets")}
    cnts = {}
    for t, seg in segs.items():
        v = (seg >= 0) & (seg < B)
        cnts[t] = np.bincount(seg[v], minlength=B)
    active = (cnts["cells"] > 0) & (cnts["tracks"] > 0) & (cnts["subjets"] > 0)
    act_ids = np.nonzero(active)[0]
    gids_per_core = [act_ids[c::N_CORES] for c in range(N_CORES)]
    G = max(8, ((max(len(g) for g in gids_per_core) + 7) // 8) * 8)

    tp = TypePrep("p", "p", [cc[:, 0], cc[:, 1],
                             np.asarray(inputs["cells_type"], np.float32),
                             np.asarray(inputs["cells_E"], np.float32)],
                  segs["cells"], B, gids_per_core)
    tt = TypePrep("t", "t", [tcn[:, 0], tcn[:, 1],
                             np.asarray(inputs["tracks_D0"], np.float32),
                             np.asarray(inputs["tracks_E"], np.float32)],
                  segs["tracks"], B, gids_per_core)
    to = TypePrep("o", "o", [sc[:, 0], sc[:, 1],
                             np.asarray(inputs["subjets_E"], np.float32)],
                  segs["subjets"], B, gids_per_core)
    return B, act_ids, gids_per_core, G, (tp, tt, to)


def _pack_weights(params, cfg):
    """Arrange all lhsT blocks into one [128, WC] f32 array + column offsets."""
    cols = []
    offs = {}

    def add(name, arr):
        a = np.zeros((128, arr.shape[1]), np.float32)
        a[:arr.shape[0], :] = arr
        offs[name] = (sum(c.shape[1] for c in cols), arr.shape[1], arr.shape[0])
        cols.append(a)

    for t, ip_key, hid_key in (("p", "init_p", "hid_p"),
                               ("t", "init_t", "hid_t"),
                               ("o", "init_o", "hid_o")):
        F = cfg["F"][t]
        ip = params[ip_key]
        add(f"{t}_i1", _inf_blk(ip["l1"]["W"], F))
        add(f"{t}_i2", _blkdiag(ip["l2"]["W"]))
        for m in range(NMOD):
            hp = params[hid_key][m]
            W0 = np.asarray(hp["l0"]["W"], np.float32)
            add(f"{t}_m{m}_inf", _inf_blk(W0[:F], F))
            add(f"{t}_m{m}_s", _blkdiag(W0[F:F + 16]))
            add(f"{t}_m{m}_m", _blkdiag(W0[F + 16:F + 32]))
            add(f"{t}_m{m}_mx", _blkdiag(W0[F + 32:F + 48]))
            add(f"{t}_m{m}_h", _blkdiag(W0[F + 48:F + 64]))
            add(f"{t}_m{m}_l1", _blkdiag(hp["l1"]["W"]))
            add(f"{t}_m{m}_l2", _blkdiag(hp["l2"]["W"]))
    c = params["clf"]
    add("c_l1", np.asarray(c["l1"]["W"], np.float32))
    add("c_l2", np.asarray(c["l2"]["W"], np.float32))
    add("c_l3", np.asarray(c["l3"]["W"], np.float32))
    add("c_l4", np.asarray(c["l4"]["W"], np.float32))
    return np.concatenate(cols, axis=1), offs


def _type_tables(params, cfg, t, ip_key, hid_key, G, ginv_g, cnt):
    """[128, 3*Gc... ] per-type table: ginv_rep | padcnt_rep | cntinv_rep | biases."""
    L = cfg["L"][t]
    F = cfg["F"][t]
    Gc = G // 8
    erow = 8 * (F - 1)
    # ginv block: multiplicative — 1.0 on non-E plane rows, 1/glob on E rows,
    # so the transform can run on the 32-aligned [0:8F] partition range.
    ginv_blk = np.ones((128, Gc), np.float32)
    ginv_blk[erow:erow + 8, :] = ginv_g.reshape(8, Gc)
    pc = (L - cnt).astype(np.float32).reshape(8, Gc)
    padcnt = np.repeat(pc, 16, axis=0)  # row 16c+f = chunk c
    with np.errstate(divide="ignore"):
        ci = np.where(cnt > 0, 1.0 / np.maximum(cnt, 1), 0.0).astype(np.float32)
    cntinv = np.repeat(ci.reshape(8, Gc), 16, axis=0)
    ip, hps = params[ip_key], params[hid_key]
    bcols = [_b_rep(ip["l1"]["b"]), _b_rep(ip["l2"]["b"])]
    for m in range(NMOD):
        bcols += [_b_rep(hps[m]["l0"]["b"]), _b_rep(hps[m]["l1"]["b"]),
                  _b_rep(hps[m]["l2"]["b"])]
    # per-partition transform vector: cells subtract 2.5 on type rows;
    # tracks multiply -1 on D0 rows (for |x| = max(x, -x)); else identity.
    tvec = np.zeros((128, 1), np.float32)
    if t == "p":
        tvec[16:24, 0] = 2.5
    elif t == "t":
        tvec[0:32, 0] = 1.0
        tvec[16:24, 0] = -1.0
    return np.concatenate([ginv_blk, padcnt, cntinv] + bcols + [tvec], axis=1)


# ----------------------------------------------------------------------------
# bass program
# ----------------------------------------------------------------------------

def _bc_ap(t_ap, parts, inner):
    """Custom AP from an existing sliced AP: partition dim + given free dims."""
    return bass.AP(tensor=t_ap.tensor, offset=t_ap.offset,
                   ap=[parts] + inner)


def _build_program(cfg):
    G = cfg["G"]
    Gc = G // 8
    alphas = cfg["alphas"]

    nc = bass.Bass()
    ext = {}
    for t in ("p", "t", "o"):
        C = Gc * cfg["L"][t]
        ext[f"raw_{t}"] = nc.dram_tensor(f"raw_{t}", [8 * cfg["F"][t], C], F32,
                                         kind="ExternalInput")
        ext[f"tab_{t}"] = nc.dram_tensor(f"tab_{t}", [128, 3 * Gc + 12], F32,
                                         kind="ExternalInput")
    ext["wts"] = nc.dram_tensor("wts", [128, cfg["WC"]], F32, kind="ExternalInput")
    ext["tabc"] = nc.dram_tensor("tabc", [128, 4], F32, kind="ExternalInput")
    out_ext = nc.dram_tensor("out", [G], F32, kind="ExternalOutput")

    offs = cfg["offs"]

    with tile.TileContext(nc) as tc:
        with tc.tile_pool(name="wpool", bufs=1) as wpool, \
             tc.tile_pool(name="rawp", bufs=1) as rawp, \
             tc.tile_pool(name="hpool", bufs=1) as hpool, \
             tc.tile_pool(name="tabp", bufs=1) as tabp, \
             tc.tile_pool(name="stat", bufs=10) as statp, \
             tc.tile_pool(name="zq", bufs=3) as zqp, \
             tc.tile_pool(name="tneg", bufs=1) as tnegp, \
             tc.tile_pool(name="mean", bufs=1) as meanp, \
             tc.tile_pool(name="clf", bufs=1) as clfp, \
             tc.tile_pool(name="zps", bufs=6, space="PSUM") as zps, \
             tc.tile_pool(name="qps", bufs=2, space="PSUM") as qps:

            wt = wpool.tile([128, cfg["WC"]], F32)
            nc.sync.dma_start(out=wt[:], in_=ext["wts"][:])
            tabc = clfp.tile([128, 4], F32, tag="tabc")
            nc.sync.dma_start(out=tabc[:], in_=ext["tabc"][:])

            def W(name):
                o, m, k = offs[name]
                return wt[0:k, o:o + m]

            means = {}

            for t in ("p", "t", "o"):
                L = cfg["L"][t]
                F = cfg["F"][t]
                C = Gc * L
                KF = 8 * F
                raw = rawp.tile([KF, C], F32, tag=f"raw{t}")
                nc.sync.dma_start(out=raw[:], in_=ext[f"raw_{t}"][:])
                tab = tabp.tile([128, 3 * Gc + 12], F32, tag=f"tab{t}")
                nc.sync.dma_start(out=tab[:], in_=ext[f"tab_{t}"][:])

                def bias(i):
                    return tab[:, 3 * Gc + i:3 * Gc + i + 1]

                tvec = tab[0:KF, 3 * Gc + 11:3 * Gc + 12]
                # infeat transforms on the full (32-aligned) [0:KF] range;
                # the per-partition vectors are identity on untouched planes
                if t == "p":
                    nc.vector.tensor_scalar(out=raw[:], in0=raw[:],
                                            scalar1=tvec, scalar2=None,
                                            op0=mybir.AluOpType.subtract)
                elif t == "t":
                    tneg = tnegp.tile([32, C], F32, tag="tneg")
                    nc.vector.tensor_scalar(out=tneg[0:KF, :], in0=raw[:],
                                            scalar1=tvec, scalar2=None,
                                            op0=mybir.AluOpType.mult)
                    nc.vector.tensor_tensor(out=raw[:], in0=raw[:],
                                            in1=tneg[0:KF, :],
                                            op=mybir.AluOpType.max)
                ginv_slice = tab[0:KF, 0:Gc]
                ginv_ap = _bc_ap(ginv_slice, ginv_slice.ap[0], [[1, Gc], [0, L]])
                rawv = raw[:].rearrange("p (g l) -> p g l", g=Gc)
                nc.vector.tensor_tensor(out=rawv, in0=rawv, in1=ginv_ap,
                                        op=mybir.AluOpType.mult)

                h = hpool.tile([128, C], F32, tag=f"h{t}")

                # column windows: (col0, width, q_ap_builder)
                windows = []
                if L > PSUM_COLS:
                    npart = (L + PSUM_COLS - 1) // PSUM_COLS
                    for k in range(Gc):
                        for pi in range(npart):
                            c0 = k * L + pi * PSUM_COLS
                            w = min(PSUM_COLS, (k + 1) * L - c0)
                            windows.append((c0, w, k, 1))
                else:
                    gpg = max(1, PSUM_COLS // L)
                    for k0 in range(0, Gc, gpg):
                        gw = min(gpg, Gc - k0)
                        windows.append((k0 * L, gw * L, k0, gw))

                # init MLP
                for (c0, w, k0, gw) in windows:
                    cs = slice(c0, c0 + w)
                    z = zps.tile([128, PSUM_COLS], F32, space="PSUM", tag="z")
                    nc.tensor.matmul(z[:, :w], lhsT=W(f"{t}_i1"), rhs=raw[:, cs],
                                     start=True, stop=True)
                    nc.scalar.activation(h[:, cs], z[:, :w],
                                         mybir.ActivationFunctionType.Prelu,
                                         bias=bias(0), alpha=alphas[f"{t}_i1"])
                    z2 = zps.tile([128, PSUM_COLS], F32, space="PSUM", tag="z")
                    nc.tensor.matmul(z2[:, :w], lhsT=W(f"{t}_i2"), rhs=h[:, cs],
                                     start=True, stop=True)
                    nc.scalar.activation(h[:, cs], z2[:, :w],
                                         mybir.ActivationFunctionType.Prelu,
                                         bias=bias(1), alpha=alphas[f"{t}_i2"])

                hv = h[:].rearrange("p (g l) -> p g l", g=Gc)
                h_first = _bc_ap(h[:], h[:].ap[0], [[L, Gc]])
                padcnt_ap = tab[:, Gc:2 * Gc]
                cntinv_ap = tab[:, 2 * Gc:3 * Gc]

                def seg_stats(tag):
                    s_pad = statp.tile([128, Gc], F32, tag="sp")
                    nc.vector.tensor_reduce(out=s_pad[:], in_=hv,
                                            axis=mybir.AxisListType.X,
                                            op=mybir.AluOpType.add)
                    t1 = statp.tile([128, Gc], F32, tag="t1")
                    nc.vector.tensor_tensor(out=t1[:], in0=padcnt_ap, in1=h_first,
                                            op=mybir.AluOpType.mult)
                    s = statp.tile([128, Gc], F32, tag="s")
                    nc.vector.tensor_tensor(out=s[:], in0=s_pad[:], in1=t1[:],
                                            op=mybir.AluOpType.subtract)
                    mn = statp.tile([128, Gc], F32, tag="mn")
                    nc.vector.tensor_tensor(out=mn[:], in0=s[:], in1=cntinv_ap,
                                            op=mybir.AluOpType.mult)
                    return s, mn

                for m in range(NMOD):
                    s, mn = seg_stats(f"{t}{m}")
                    mx = statp.tile([128, Gc], F32, tag="mx")
                    nc.vector.tensor_reduce(out=mx[:], in_=hv,
                                            axis=mybir.AxisListType.X,
                                            op=mybir.AluOpType.max)
                    qp = qps.tile([128, Gc], F32, space="PSUM", tag="q")
                    nc.tensor.matmul(qp[:], lhsT=W(f"{t}_m{m}_s"), rhs=s[:],
                                     start=True, stop=False)
                    nc.tensor.matmul(qp[:], lhsT=W(f"{t}_m{m}_m"), rhs=mn[:],
                                     start=False, stop=False)
                    nc.tensor.matmul(qp[:], lhsT=W(f"{t}_m{m}_mx"), rhs=mx[:],
                                     start=False, stop=True)
                    q = statp.tile([128, Gc], F32, tag="qsb")
                    nc.vector.tensor_scalar(out=q[:], in0=qp[:],
                                            scalar1=bias(2 + 3 * m), scalar2=None,
                                            op0=mybir.AluOpType.add)

                    for (c0, w, k0, gw) in windows:
                        cs = slice(c0, c0 + w)
                        z = zps.tile([128, PSUM_COLS], F32, space="PSUM", tag="z")
                        nc.tensor.matmul(z[:, :w], lhsT=W(f"{t}_m{m}_inf"),
                                         rhs=raw[:, cs], start=True, stop=False)
                        nc.tensor.matmul(z[:, :w], lhsT=W(f"{t}_m{m}_h"),
                                         rhs=h[:, cs], start=False, stop=True)
                        if L > PSUM_COLS:
                            # single-graph window: q broadcast = per-partition
                            # ACT bias — no staging, no DVE add
                            nc.scalar.activation(h[:, cs], z[:, :w],
                                                 mybir.ActivationFunctionType.Prelu,
                                                 bias=q[:, k0:k0 + 1],
                                                 alpha=alphas[f"{t}_m{m}_a0"])
                        else:
                            q_ap = _bc_ap(q[:, k0:k0 + gw], q[:].ap[0],
                                          [[1, gw], [0, L]])
                            zview = z[:, :w].rearrange("p (g l) -> p g l", g=gw)
                            zq = zqp.tile([128, PSUM_COLS], F32, tag="zq")
                            zqw = zq[:, :w].rearrange("p (g l) -> p g l", g=gw)
                            nc.vector.tensor_tensor(out=zqw, in0=zview, in1=q_ap,
                                                    op=mybir.AluOpType.add)
                            nc.scalar.activation(h[:, cs], zq[:, :w],
                                                 mybir.ActivationFunctionType.Prelu,
                                                 bias=0.0,
                                                 alpha=alphas[f"{t}_m{m}_a0"])
                        z1 = zps.tile([128, PSUM_COLS], F32, space="PSUM", tag="z")
                        nc.tensor.matmul(z1[:, :w], lhsT=W(f"{t}_m{m}_l1"),
                                         rhs=h[:, cs], start=True, stop=True)
                        nc.scalar.activation(h[:, cs], z1[:, :w],
                                             mybir.ActivationFunctionType.Prelu,
                                             bias=bias(3 + 3 * m),
                                             alpha=alphas[f"{t}_m{m}_a1"])
                        z2 = zps.tile([128, PSUM_COLS], F32, space="PSUM", tag="z")
                        nc.tensor.matmul(z2[:, :w], lhsT=W(f"{t}_m{m}_l2"),
                                         rhs=h[:, cs], start=True, stop=True)
                        nc.scalar.activation(h[:, cs], z2[:, :w],
                                             mybir.ActivationFunctionType.Prelu,
                                             bias=bias(4 + 3 * m),
                                             alpha=alphas[f"{t}_m{m}_a2"])

                # final means
                s, mn = seg_stats(f"{t}_fin")
                mt = meanp.tile([128, Gc], F32, tag=f"mean{t}")
                nc.vector.tensor_copy(out=mt[:], in_=mn[:])
                means[t] = mt

            # classifier: assemble [48, G] via SBUF->SBUF DMAs (partition moves)
            xc = clfp.tile([128, G], F32, tag="xc")
            for fb, t in ((0, "p"), (16, "o"), (32, "t")):
                for c in range(8):
                    nc.sync.dma_start(
                        out=xc[fb:fb + 16, c * Gc:(c + 1) * Gc],
                        in_=means[t][16 * c:16 * c + 16, :])

            def clf_bias(i, rows):
                return tabc[0:rows, i:i + 1]

            zc = zps.tile([128, PSUM_COLS], F32, space="PSUM", tag="z")
            nc.tensor.matmul(zc[0:48, :G], lhsT=W("c_l1"), rhs=xc[0:48, :G],
                             start=True, stop=True)
            x1 = clfp.tile([128, G], F32, tag="x1")
            nc.scalar.activation(x1[0:48, :G], zc[0:48, :G],
                                 mybir.ActivationFunctionType.Prelu,
                                 bias=clf_bias(0, 48), alpha=alphas["c_a1"])
            zc2 = zps.tile([128, PSUM_COLS], F32, space="PSUM", tag="z")
            nc.tensor.matmul(zc2[0:32, :G], lhsT=W("c_l2"), rhs=x1[0:48, :G],
                             start=True, stop=True)
            x2 = clfp.tile([128, G], F32, tag="x2")
            nc.scalar.activation(x2[0:32, :G], zc2[0:32, :G],
                                 mybir.ActivationFunctionType.Prelu,
                                 bias=clf_bias(1, 32), alpha=alphas["c_a2"])
            zc3 = zps.tile([128, PSUM_COLS], F32, space="PSUM", tag="z")
            nc.tensor.matmul(zc3[0:16, :G], lhsT=W("c_l3"), rhs=x2[0:32, :G],
                             start=True, stop=True)
            x3 = clfp.tile([128, G], F32, tag="x3")
            nc.scalar.activation(x3[0:16, :G], zc3[0:16, :G],
                                 mybir.ActivationFunctionType.Prelu,
                                 bias=clf_bias(2, 16), alpha=alphas["c_a3"])
            zc4 = zps.tile([128, PSUM_COLS], F32, space="PSUM", tag="z")
            nc.tensor.matmul(zc4[0:1, :G], lhsT=W("c_l4"), rhs=x3[0:16, :G],
                             start=True, stop=True)
            yo = clfp.tile([128, G], F32, tag="yo")
            nc.scalar.activation(yo[0:1, :G], zc4[0:1, :G],
                                 mybir.ActivationFunctionType.Identity,
                                 bias=clf_bias(3, 1))
            nc.sync.dma_start(out=out_ext[:], in_=yo[0:1, :G])

    _legalize_multiwait(nc)
    return nc


def _legalize_multiwait(nc):
    """Walrus CoreV3 codegen accepts only one sync wait per instruction; move
    extra waits onto same-engine NOPs placed immediately before."""
    import bass_rust
    for bb in nc.main_func.blocks:
        insts = list(bb.instructions)
        new_insts = []
        changed = False
        for inst in insts:
            si = inst.sync_info
            if si is not None and si.on_wait and len(si.on_wait) > 1:
                waits = list(si.on_wait)
                for k, w in enumerate(waits[:-1]):
                    nop = bass_rust.InstNoOp(name=f"{inst.name}_wsplit{k}")
                    nop.engine = inst.engine
                    nop.sync_info = mybir.SyncInfo(on_wait=[w], on_update=[])
                    new_insts.append(nop)
                inst.sync_info = mybir.SyncInfo(on_wait=[waits[-1]],
                                                on_update=list(si.on_update))
                changed = True
            new_insts.append(inst)
        if changed:
            bb.instructions.clear()
            bb.instructions.extend(new_insts)


# ----------------------------------------------------------------------------
# entry point
# ----------------------------------------------------------------------------

def kernel(**inputs):
    B, act_ids, gids_per_core, G, (tp, tt, to) = _host_prep(inputs)
    params = inputs["params"]
    glob = np.asarray(inputs["glob_E"], np.float32)
    ginv = (1.0 / glob).astype(np.float32)
    Gc = G // 8

    cfg = {
        "G": G,
        "L": {"p": tp.L, "t": tt.L, "o": to.L},
        "F": {"p": 4, "t": 4, "o": 3},
        "alphas": {},
    }
    for t, ipk, hk in (("p", "init_p", "hid_p"), ("t", "init_t", "hid_t"),
                       ("o", "init_o", "hid_o")):
        ip = params[ipk]
        cfg["alphas"][f"{t}_i1"] = float(np.asarray(ip["a1"]))
        cfg["alphas"][f"{t}_i2"] = float(np.asarray(ip["a2"]))
        for m in range(NMOD):
            hp = params[hk][m]
            for a in ("a0", "a1", "a2"):
                cfg["alphas"][f"{t}_m{m}_{a}"] = float(np.asarray(hp[a]))
    for a in ("a1", "a2", "a3"):
        cfg["alphas"][f"c_{a}"] = float(np.asarray(params["clf"][a]))

    wts, offs = _pack_weights(params, cfg)
    cfg["WC"] = wts.shape[1]
    cfg["offs"] = offs

    key = (G, tp.L, tt.L, to.L, tuple(sorted(cfg["alphas"].items())))
    if key not in _PROGRAM_CACHE:
        _PROGRAM_CACHE[key] = _build_program(cfg)
    nc = _PROGRAM_CACHE[key]

    tabc = np.concatenate([
        np.pad(_brep_n(params["clf"]["l1"]["b"]), ((0, 80), (0, 0))),
        np.pad(_brep_n(params["clf"]["l2"]["b"]), ((0, 96), (0, 0))),
        np.pad(_brep_n(params["clf"]["l3"]["b"]), ((0, 112), (0, 0))),
        np.pad(_brep_n(params["clf"]["l4"]["b"]), ((0, 127), (0, 0))),
    ], axis=1)

    in_maps = []
    for c in range(N_CORES):
        gids = np.asarray(gids_per_core[c], np.int64)
        ginv_g = np.zeros(G, np.float32)
        if len(gids):
            ginv_g[:len(gids)] = ginv[gids]
        m = {"wts": wts, "tabc": tabc}
        for t, prep, ipk, hk in (("p", tp, "init_p", "hid_p"),
                                 ("t", tt, "init_t", "hid_t"),
                                 ("o", to, "init_o", "hid_o")):
            idx, cnt = prep.core_tables(gids, G)
            m[f"raw_{t}"] = prep.raw_packed(idx, G)
            m[f"tab_{t}"] = _type_tables(params, cfg, t, ipk, hk, G, ginv_g, cnt)
        in_maps.append(m)

    import os
    global LAST_EXEC_NS
    trace = os.environ.get("DEEPSET_TRACE", "") == "1"
    kw = {}
    if trace:
        os.environ["BASS_PERFETTO_PROFILE_ALL_CORES"] = "1"
        kw = dict(trace=True, trace_cores=list(range(N_CORES)))
    res = run_bass_kernel_spmd(nc, in_maps, core_ids=list(range(N_CORES)), **kw)
    LAST_EXEC_NS = res.exec_time_ns
    globals()["LAST_RESULT"] = res

    out = np.full((B, 1), np.nan, np.float32)
    for c in range(N_CORES):
        gids = gids_per_core[c]
        out[gids, 0] = res.results[c]["out"][:len(gids)]
    return out


def _brep_n(b):
    b = np.asarray(b, np.float32)
    return b[:, None]
